# revision 40
# baseline (speedup 1.0000x reference)
"""Multi-head attention Trainium2 kernel.

B=8, S=1024, D=1024, H=16 heads, head_dim=64.
Sharding: pure data parallelism over batch — one batch element per
NeuronCore, weights replicated, no collectives.

Per-core dataflow (all matmul operands bf16, fp32 PSUM accumulate):
  host:   xT = x.T (k-major) for q/k/v, WT = W.T for all weights, bf16.
  QT[do,s] = sum_k WqT[k,do]*xqT[k,s]   (+bq via DVE per-partition add)
  KT[do,s] likewise
  V[s,dv]  = sum_k xvT[k,s]*WvT[k,dv]   (+bv via K=1 ones-row matmul)
             scattered into V65 layout: per head pair [V_e(64)|1|1|V_o(64)]
  per (head-pair p, i-chunk c):
    for j-tile: SEO[:, 0:512]  = KT_h[0:64,j].T  @ QT_h[0:64,i]   (rows 0-63)
                SEO[:, 512:1K] = KT_h[64:,j].T   @ QT_h[64:,i]    (rows 64-127)
                exp(SEO/8) via one ACT -> expEO (bf16 SBUF)
    AV: psum[j->] = V65_slice.T @ expEO slices -> OT rows + colsum row
    normalize: DVE recip_approx_fast + K=1 PE broadcast + DVE mul -> OT bf16
  out[s,do] = sum_dv OT[dv,s]*WoT[dv,do] (+bo via ones-row) -> fp32 -> DRAM
"""

import numpy as np
import ml_dtypes
from contextlib import ExitStack

import concourse.bass as bass
import concourse.tile as tile
import concourse.mybir as mybir
from concourse import bacc
from concourse.bass_utils import run_bass_kernel_spmd

BF16 = mybir.dt.bfloat16
F32 = mybir.dt.float32
AF = mybir.ActivationFunctionType

S = 1024
D = 1024
H = 16
HD = 64
P = 128
KT = D // P      # 8 contraction tiles
MT = S // P      # 8 row tiles
NC = 512         # free-dim chunk (one PSUM bank of fp32)
NCH = S // NC    # 2 chunks
PAIRS = H // 2   # 8
VW = 160  # per-pair V65 width [V_e(64) | ones@64 | gap 65-95 | V_o@96-159]
N_CORES = 8


def build_body(ctx: ExitStack, tc, io, dbg=None):
    nc = tc.nc

    const = ctx.enter_context(tc.tile_pool(name="const", bufs=1))
    qkt = ctx.enter_context(tc.tile_pool(name="qkt", bufs=1))
    v65p = ctx.enter_context(tc.tile_pool(name="v65", bufs=1))
    otp = ctx.enter_context(tc.tile_pool(name="otp", bufs=1))
    xw = ctx.enter_context(tc.tile_pool(name="xw", bufs=2))
    wop = ctx.enter_context(tc.tile_pool(name="wop", bufs=1))
    expp = ctx.enter_context(tc.tile_pool(name="expp", bufs=2))
    sb = ctx.enter_context(tc.tile_pool(name="sb", bufs=1))
    osbp = ctx.enter_context(tc.tile_pool(name="osb", bufs=3))
    # PSUM: psS 2x[128,1024] (4 banks) scores + projection chains,
    #       psAV 2x[128,512] AV pairs, psR 2x[128,512] recip-bcast + outproj.
    psS = ctx.enter_context(tc.tile_pool(name="psS", bufs=2, space="PSUM"))
    psAV = ctx.enter_context(tc.tile_pool(name="psAV", bufs=2, space="PSUM"))
    psR = ctx.enter_context(tc.tile_pool(name="psR", bufs=2, space="PSUM"))

    def dump_sbuf(key, tile_ap, row0=0):
        if dbg is not None and key in dbg:
            nc.gpsimd.dma_start(
                dbg[key][row0 : row0 + tile_ap.shape[0], :], tile_ap)

    # ---- HAM warm-up first: dummy matmuls (zero inputs, discarded output)
    # keep the PE busy during the initial input DMA (~11 us) so the real
    # projections start at the warm 2.4 GHz clock. Self-contained dep:
    # one memset, then back-to-back matmuls.
    warm = const.tile([P, NC], BF16, tag="warm")
    nc.vector.memset(warm[:], 0.0)
    for _ in range(40):
        wps = psR.tile([P, NC], F32, tag="r", name="warmps")
        nc.tensor.matmul(wps[:], warm[:, 0:P], warm[:], start=True, stop=True)

    # ---- constants ----
    ones = const.tile([1, P], BF16, tag="ones")
    nc.vector.memset(ones[:], 1.0)
    ones128 = const.tile([P, P], BF16, tag="ones128")
    nc.vector.memset(ones128[:], 1.0)
    bqc = const.tile([P, KT], F32, tag="bqc")
    nc.sync.dma_start(bqc[:], io["bq_c"][:])
    bkc = const.tile([P, KT], F32, tag="bkc")
    nc.sync.dma_start(bkc[:], io["bk_c"][:])
    bvr = const.tile([1, PAIRS * VW], BF16, tag="bvr")
    nc.sync.dma_start(bvr[:], io["bv65_r"][:])
    bor = const.tile([1, D], BF16, tag="bor")
    nc.sync.dma_start(bor[:], io["bo_r"][:])

    # Bias rows broadcast to all 128 partitions once (K=1 ones matmuls,
    # also part of the PE warm-up); V/O bias then rides the PSUM->SBUF
    # evacuation as a DVE add instead of 32 per-chunk bias matmuls.
    bv65t = const.tile([P, PAIRS * VW], BF16, tag="bv65t")
    bot = const.tile([P, D], BF16, tag="bot")
    for lo in range(0, PAIRS * VW, NC):
        hi = min(lo + NC, PAIRS * VW)
        bps = psR.tile([P, NC], F32, tag="r", name="bcastps")
        nc.tensor.matmul(bps[:, 0 : hi - lo], ones[0:1, :], bvr[0:1, lo:hi],
                         start=True, stop=True)
        nc.vector.tensor_copy(bv65t[:, lo:hi], bps[:, 0 : hi - lo])
    for lo in range(0, D, NC):
        bps = psR.tile([P, NC], F32, tag="r", name="bcastps")
        nc.tensor.matmul(bps[:], ones[0:1, :], bor[0:1, lo : lo + NC],
                         start=True, stop=True)
        nc.vector.tensor_copy(bot[:, lo : lo + NC], bps[:])

    # ---- persistent activation tiles ----
    QT = [qkt.tile([P, S], BF16, tag=f"qt{m}", name=f"qt{m}") for m in range(MT)]
    KTt = [qkt.tile([P, S], BF16, tag=f"kt{m}", name=f"ktt{m}") for m in range(MT)]
    V65 = [v65p.tile([P, PAIRS * VW], BF16, tag=f"v65_{m}", name=f"v65_{m}") for m in range(MT)]
    OT = [otp.tile([P, S], BF16, tag=f"ot{m}", name=f"ot{m}") for m in range(MT)]

    # ones column at col 64 of each 160-wide pair block; zero the gap
    for m in range(MT):
        v = V65[m].rearrange("p (pr w) -> p pr w", w=VW)
        nc.vector.memset(v[:, :, HD : HD + 1], 1.0)
        nc.vector.memset(v[:, :, HD + 1 : 96], 0.0)

    def load_xw(xdram, wdram):
        xt = [xw.tile([P, S], BF16, tag=f"x{k}", name=f"xt{k}") for k in range(KT)]
        wt = [xw.tile([P, D], BF16, tag=f"w{k}", name=f"wt{k}") for k in range(KT)]
        for k in range(KT):
            nc.sync.dma_start(xt[k][:], xdram[k * P : (k + 1) * P, :])
            nc.sync.dma_start(wt[k][:], wdram[k * P : (k + 1) * P, :])
        return xt, wt

    # Projection chains use the psR banks ([128,512] granularity) so the
    # psS score tiles and psAV attention banks stay free — attention can
    # begin the moment KTt[0]/QT[0] land, without waiting out the
    # proj-chain pool rotation.
    def proj_ps(i):
        return psR.tile([P, NC], F32, tag="r", name="psproj")

    # ---------- V projection (first: V65 needed by all attention) ----------
    xtv, wtv = load_xw(io["xvT"], io["wvT"])
    for m in range(MT):
        for c in range(NCH):     # V chain (m, c): dv chunk
            ps = proj_ps(2 * m + c)
            for k in range(KT):
                nc.tensor.matmul(
                    ps[:],
                    xtv[k][:, m * P : (m + 1) * P],
                    wtv[k][:, c * NC : (c + 1) * NC],
                    start=(k == 0),
                    stop=(k == KT - 1),
                )
            # scatter heads into V65 pair layout, fusing the bias add
            psv = ps.rearrange("p (pr two x) -> p pr two x", two=2, x=HD)
            v = V65[m].rearrange("p (pr w) -> p pr w", w=VW)
            bvv = bv65t.rearrange("p (pr w) -> p pr w", w=VW)
            pr0 = c * (NC // (2 * HD))
            npr = NC // (2 * HD)
            nc.vector.tensor_add(
                v[:, pr0 : pr0 + npr, 0:HD], psv[:, :, 0, :],
                bvv[:, pr0 : pr0 + npr, 0:HD])
            nc.vector.tensor_add(
                v[:, pr0 : pr0 + npr, 96:VW], psv[:, :, 1, :],
                bvv[:, pr0 : pr0 + npr, 96:VW])

    # ---------- K and Q projections (interleaved with attention below) ----
    xtk, wtk = load_xw(io["xkT"], io["wkT"])
    xtq, wtq = load_xw(io["xqT"], io["wqT"])

    # WoT loads overlap attention (wop pool is free from the start)
    wo_t = [wop.tile([P, D], BF16, tag=f"wo{k}", name=f"wo{k}") for k in range(KT)]
    for k in range(KT):
        nc.sync.dma_start(wo_t[k][:], io["woT"][k * P : (k + 1) * P, :])

    def emit_kq(m):
        for xt, wt, bias, dst in ((xtk, wtk, bkc, KTt), (xtq, wtq, bqc, QT)):
            for c in range(NCH):
                ps = proj_ps(2 * m + c)
                for k in range(KT):
                    nc.tensor.matmul(
                        ps[:],
                        wt[k][:, m * P : (m + 1) * P],
                        xt[k][:, c * NC : (c + 1) * NC],
                        start=(k == 0),
                        stop=(k == KT - 1),
                    )
                nc.vector.tensor_scalar_add(
                    dst[m][:, c * NC : (c + 1) * NC], ps[:], bias[:, m : m + 1])

    # ---------- attention ----------
    def emit_scores(p, c):
        """exp(scores/8) for heads 2p/2p+1 into one [128, 8*1024] SBUF buf.

        Per j-tile: one [128,1024] PSUM tile, E scores (K=64, rows 0-63 of
        KT/QT) in cols 0:512, O scores (rows 64-127) in cols 512:1024 —
        adjacent matmuls on disjoint PE row groups, one ACT drains both.
        """
        expEO = expp.tile([P, KT * 2 * NC], BF16, tag="expEO")
        for j in range(KT):
            s = psS.tile([P, 2 * NC], F32, tag="s", name="sEO")
            nc.tensor.matmul(
                s[:, 0:NC],
                KTt[p][0:HD, j * P : (j + 1) * P],
                QT[p][0:HD, c * NC : (c + 1) * NC],
                start=True, stop=True,
            )
            nc.tensor.matmul(
                s[:, NC : 2 * NC],
                KTt[p][HD:P, j * P : (j + 1) * P],
                QT[p][HD:P, c * NC : (c + 1) * NC],
                start=True, stop=True,
            )
            nc.scalar.activation(
                expEO[:, j * 2 * NC : (j + 1) * 2 * NC], s[:], AF.Exp,
                scale=0.125)
        if p == 0 and c == 0:
            dump_sbuf("expEO", expEO[:])
        return expEO

    def emit_av(p, c, expEO):
        if not hasattr(emit_av, "seen"):
            emit_av.seen = 0
        avE = psAV.tile([P, NC], F32, tag="av", name="avE")
        avO = psAV.tile([P, NC], F32, tag="av", name="avO")
        for jt in range(KT):
            nc.tensor.matmul(
                avE[:], V65[jt][:, p * VW : p * VW + P],
                expEO[:, jt * 2 * NC : jt * 2 * NC + NC],
                start=(jt == 0), stop=(jt == KT - 1),
            )
            nc.tensor.matmul(
                avO[:], V65[jt][:, p * VW + 32 : p * VW + 32 + P],
                expEO[:, jt * 2 * NC + NC : (jt + 1) * 2 * NC],
                start=(jt == 0), stop=(jt == KT - 1),
            )
        # avE rows: 0-63 = OT_even, 64 = colsum_even
        # avO rows: 32 = colsum_odd, 64-127 = OT_odd
        rdn = sb.tile([P, NC], F32, tag="denom")
        rcf = sb.tile([P, NC], F32, tag="recipf")
        rc = sb.tile([P, NC], BF16, tag="recip")
        if emit_av.seen < 2:  # first use of each rdn buf: fill unused lanes
            emit_av.seen += 1
            nc.vector.memset(rdn[:], 1.0)
        # stage denom rows PSUM->SBUF; recip_approx_fast is only correct on
        # full-partition base-0 APs, so run it on the whole tile (same
        # duration — lanes are independent; only rows 64/32 are consumed).
        nc.vector.tensor_copy(rdn[HD : HD + 1, :], avE[HD : HD + 1, :])
        nc.vector.tensor_copy(rdn[32:33, :], avO[32:33, :])
        nc.vector.reciprocal_approx_fast(rcf[:], rdn[:])
        nc.vector.tensor_copy(rc[HD : HD + 1, :], rcf[HD : HD + 1, :])
        nc.vector.tensor_copy(rc[32:33, :], rcf[32:33, :])
        # broadcast each recip row to all 128 partitions via K=1 PE matmul
        rpsE = psR.tile([P, NC], F32, tag="r", name="rpsE")
        rpsO = psR.tile([P, NC], F32, tag="r", name="rpsO")
        nc.tensor.matmul(
            rpsE[:], ones128[HD : HD + 1, :], rc[HD : HD + 1, :],
            start=True, stop=True,
        )
        nc.tensor.matmul(
            rpsO[:], ones128[32:33, :], rc[32:33, :],
            start=True, stop=True,
        )
        Rt = sb.tile([P, NC], BF16, tag="bcast")
        nc.vector.tensor_copy(Rt[0:HD, :], rpsE[0:HD, :])
        nc.vector.tensor_copy(Rt[HD:P, :], rpsO[HD:P, :])
        nc.vector.tensor_mul(
            OT[p][0:HD, c * NC : (c + 1) * NC], avE[0:HD, :], Rt[0:HD, :])
        nc.vector.tensor_mul(
            OT[p][HD:P, c * NC : (c + 1) * NC], avO[HD:P, :], Rt[HD:P, :])

    def emit_outproj(m, c):
        ps = psR.tile([P, NC], F32, tag="r", name="pso")
        for kt in range(KT):
            nc.tensor.matmul(
                ps[:],
                OT[kt][:, m * P : (m + 1) * P],
                wo_t[kt][:, c * NC : (c + 1) * NC],
                start=(kt == 0), stop=(kt == KT - 1),
            )
        osb = osbp.tile([P, NC], BF16, tag="osb")
        nc.vector.tensor_add(osb[:], ps[:], bot[:, c * NC : (c + 1) * NC])
        nc.sync.dma_start(
            io["out"][m * P : (m + 1) * P, c * NC : (c + 1) * NC], osb[:])

    # K/Q projections interleaved with attention: pair p's scores emit
    # right after K(p)/Q(p). The interleave keeps the PE oversubscribed —
    # under HAM, a PE that is ever under-filled throttles to 1.2 GHz and
    # becomes the bottleneck anyway, so dense PE beats "balanced".
    blocks = [(p, c) for c in range(NCH) for p in range(PAIRS)]
    op_chunks = [(m, cd) for m in range(MT) for cd in range(NCH)]
    pending = None
    emitted_op = 0
    bi = 0

    emit_kq(0)
    for m in range(1, MT):
        blk = blocks[bi]
        bi += 1
        e = emit_scores(*blk)
        if pending is not None:
            emit_av(pending[0][0], pending[0][1], pending[1])
        pending = (blk, e)
        emit_kq(m)

    if dbg is not None:
        for m in range(MT):
            dump_sbuf("qt", QT[m][:], m * P)
            dump_sbuf("kt", KTt[m][:], m * P)
            dump_sbuf("v65", V65[m][:], m * P)

    for i in range(bi, len(blocks)):
        blk = blocks[i]
        e = emit_scores(*blk)
        emit_av(pending[0][0], pending[0][1], pending[1])
        pending = (blk, e)
        if i >= 8:
            while emitted_op < min(i - 7, MT):
                emit_outproj(*op_chunks[emitted_op])
                emitted_op += 1
    emit_av(pending[0][0], pending[0][1], pending[1])

    if dbg is not None:
        for m in range(MT):
            dump_sbuf("ot", OT[m][:], m * P)

    # ---------- remaining output projection ----------
    for m, cd in op_chunks[emitted_op:]:
        emit_outproj(m, cd)


def declare_io(nc):
    def din(name, shape, dt):
        return nc.dram_tensor(name, shape, dt, kind="ExternalInput").ap()

    io = {
        "xqT": din("xqT", [D, S], BF16),
        "xkT": din("xkT", [D, S], BF16),
        "xvT": din("xvT", [D, S], BF16),
        "wqT": din("wqT", [D, D], BF16),
        "wkT": din("wkT", [D, D], BF16),
        "wvT": din("wvT", [D, D], BF16),
        "woT": din("woT", [D, D], BF16),
        "bq_c": din("bq_c", [P, KT], F32),
        "bk_c": din("bk_c", [P, KT], F32),
        "bv65_r": din("bv65_r", [1, PAIRS * VW], BF16),
        "bo_r": din("bo_r", [1, D], BF16),
        "out": nc.dram_tensor("out", [S, D], BF16, kind="ExternalOutput").ap(),
    }
    return io


_NC_CACHE = {}


def get_nc():
    if "nc" not in _NC_CACHE:
        nc = bacc.Bacc(
            "TRN2",
            target_bir_lowering=False,
            debug=False,
            enable_asserts=False,
            num_devices=N_CORES,
        )
        io = declare_io(nc)
        with tile.TileContext(nc) as tc:
            with ExitStack() as ctx:
                build_body(ctx, tc, io)
        nc.compile()
        _NC_CACHE["nc"] = nc
    return _NC_CACHE["nc"]


def prep_inputs(query, key, value, Wq, bq, Wk, bk, Wv, bv, Wo, bo):
    bf = ml_dtypes.bfloat16
    f32 = np.float32

    def t16(a):
        return np.ascontiguousarray(np.asarray(a, dtype=f32).T).astype(bf)

    # bv in V65 pair layout: per pair [even-head dims | pad 32 | odd-head dims]
    bv65 = np.zeros((1, PAIRS * VW), dtype=f32)
    bvf = np.asarray(bv, dtype=f32)
    for pr in range(PAIRS):
        bv65[0, pr * VW : pr * VW + HD] = bvf[pr * 2 * HD : pr * 2 * HD + HD]
        bv65[0, pr * VW + 96 : pr * VW + VW] = bvf[pr * 2 * HD + HD : (pr + 1) * 2 * HD]
    bv65 = bv65.astype(bf)

    base = {
        "wqT": t16(Wq),
        "wkT": t16(Wk),
        "wvT": t16(Wv),
        "woT": t16(Wo),
        "bq_c": np.ascontiguousarray(
            np.asarray(bq, dtype=f32).reshape(KT, P).T),
        "bk_c": np.ascontiguousarray(
            np.asarray(bk, dtype=f32).reshape(KT, P).T),
        "bv65_r": bv65,
        "bo_r": np.asarray(bo, dtype=f32).astype(bf).reshape(1, D),
    }
    in_maps = []
    for b in range(np.asarray(query).shape[0]):
        m = dict(base)
        m["xqT"] = t16(query[b])
        m["xkT"] = t16(key[b])
        m["xvT"] = t16(value[b])
        in_maps.append(m)
    return in_maps


def kernel(query, key, value, Wq, bq, Wk, bk, Wv, bv, Wo, bo, **run_kwargs):
    nc = get_nc()
    in_maps = prep_inputs(query, key, value, Wq, bq, Wk, bk, Wv, bv, Wo, bo)
    res = run_bass_kernel_spmd(
        nc, in_maps, core_ids=list(range(N_CORES)), **run_kwargs)
    out = np.stack(
        [res.results[b]["out"] for b in range(N_CORES)], axis=0
    ).astype(np.float32)
    if run_kwargs:
        kernel.last_results = res
    return out
